# revision 20
# baseline (speedup 1.0000x reference)
"""AttentionGRUCell Trainium2 kernel (8 NeuronCores, data-parallel over batch).

reference math:
  batch_H_proj = einsum('btd,hd->bth', batch_H, i2h_w)
  prev_proj    = prev_hidden @ h2h_w.T + h2h_b
  res   = tanh(batch_H_proj + prev_proj[:,None,:])
  e     = einsum('bth,oh->bto', res, score_w)
  alpha = softmax(e, axis=1);  alpha_t = transpose(alpha, (0,2,1))
  context = einsum('bot,btd->bod', alpha_t, batch_H)[:,0,:]
  concat  = [context, char_onehots]
  gi = concat @ rnn_w_ih.T + rnn_b_ih ; gh = prev_hidden @ rnn_w_hh.T + rnn_b_hh
  r = sig(i_r+h_r); z = sig(i_z+h_z); n = tanh(i_n + r*h_n)
  cur = (1-z)*n + z*prev_hidden
returns (cur_hidden [B,H], alpha_t [B,1,T])

Sharding: batch rows split 8 ways (32 rows per core), weights replicated.
No collectives. Compute dtype bf16 (f32 accumulation in PSUM).

Layout notes:
 - proj computed in [h, t] orientation so tanh(+prev_proj) fuses into one
   ScalarE activation with per-partition bias.
 - batch_H is cast f32->bf16 during the gpsimd DMA and transposed on-chip by
   the DMA xbar (dma_start_transpose). Transposed tiles are write-once
   (the xpose ISA slot supports a single sync wait).
 - HWDGE queues carry ONLY xbar transposes (all plain DMA on gpsimd/SWDGE)
   to avoid xbar-mode-transition serialization waits.
 - softmax has no max-subtraction (|e| <~ 1.5) and runs per b-pair so
   context matmuls free batch_H tiles early (rotating pool).
"""

import numpy as np

import concourse.bass as bass
import concourse.tile as tile
from concourse import mybir
from concourse.bass_utils import run_bass_kernel_spmd

B, T, D, H, E = 256, 256, 512, 512, 97
NCORES = 8
BC = B // NCORES          # 32 batch rows per core
NP = BC * T               # 8192 token rows per core
NJ = BC // 2              # 16 b-pairs
H3 = 3 * H

F32 = mybir.dt.float32
BF16 = mybir.dt.bfloat16
AF = mybir.ActivationFunctionType
AX = mybir.AxisListType


def build_nc():
    nc = bass.Bass()

    prev_d = nc.declare_dram_parameter("prev_hidden", [BC, H], F32, isOutput=False)
    bh_d = nc.declare_dram_parameter("batch_H", [NP, D], F32, isOutput=False)
    oneh_d = nc.declare_dram_parameter("char_onehots", [BC, E], F32, isOutput=False)
    i2h_d = nc.declare_dram_parameter("i2h_w", [H, D], F32, isOutput=False)
    h2h_d = nc.declare_dram_parameter("h2h_w", [H, H], F32, isOutput=False)
    h2hb_d = nc.declare_dram_parameter("h2h_b", [H], F32, isOutput=False)
    score_d = nc.declare_dram_parameter("score_w", [1, H], F32, isOutput=False)
    wih_d = nc.declare_dram_parameter("rnn_w_ih", [H3, D + E], F32, isOutput=False)
    bih_d = nc.declare_dram_parameter("rnn_b_ih", [H3], F32, isOutput=False)
    whh_d = nc.declare_dram_parameter("rnn_w_hh", [H3, H], F32, isOutput=False)
    bhh_d = nc.declare_dram_parameter("rnn_b_hh", [H3], F32, isOutput=False)
    outh_d = nc.declare_dram_parameter("out_hidden", [BC, H], F32, isOutput=True)
    outa_d = nc.declare_dram_parameter("out_alpha", [BC, T], F32, isOutput=True)

    import ml_dtypes

    eye_d = nc.inline_tensor(np.eye(128, dtype=ml_dtypes.bfloat16), name="eye128")

    with tile.TileContext(nc) as tc:
        with (
            tc.tile_pool(name="persist", bufs=1) as pp,
            tc.tile_pool(name="bhn", bufs=1) as bhn_pool,
            tc.tile_pool(name="wnat", bufs=3) as wnat_pool,
            tc.tile_pool(name="res", bufs=3) as res_pool,
            tc.tile_pool(name="rows", bufs=3) as row_pool,
            tc.tile_pool(name="soft", bufs=2) as soft_pool,
            tc.tile_pool(name="gate", bufs=4) as gate_pool,
            tc.tile_pool(name="ps", bufs=2, space=bass.MemorySpace.PSUM) as ps,
        ):
            # ---------- constants / small loads (all SWDGE) ----------
            eye_b = pp.tile([128, 128], BF16, tag="eye_b")
            nc.gpsimd.dma_start(eye_b[:], eye_d[:, :])
            ones_b = pp.tile([1, BC], BF16, tag="ones_b")
            nc.vector.memset(ones_b[:], 1.0)
            scoreT = pp.tile([128, 4], BF16, tag="scoreT")
            nc.gpsimd.dma_start(
                scoreT[:], score_d[:, :].rearrange("o (c p) -> p (o c)", p=128)
            )
            h2hb_pc = pp.tile([128, 4], F32, tag="h2hb")
            nc.gpsimd.dma_start(h2hb_pc[:], h2hb_d[:].rearrange("(c p) -> p c", p=128))
            bih_row = pp.tile([1, H3], BF16, tag="bihrow")
            nc.gpsimd.dma_start(bih_row[:], bih_d[:])
            bhh_row = pp.tile([1, H3], BF16, tag="bhhrow")
            nc.gpsimd.dma_start(bhh_row[:], bhh_d[:])
            prev_f = pp.tile([BC, H], F32, tag="prev_f")
            nc.gpsimd.dma_start(prev_f[:], prev_d[:, :])
            prev_b = pp.tile([BC, H], BF16, tag="prev_b")
            nc.gpsimd.dma_start(prev_b[:], prev_d[:, :])
            oneh_b = pp.tile([BC, E], BF16, tag="oneh_b")
            nc.gpsimd.dma_start(oneh_b[:], oneh_d[:, :])

            # ---------- weights: load bf16 natural (unique tiles) + transpose ----------
            # i2h^T/h2h^T: [128, 2048], block (kc,hc) at cols 512*kc+128*hc
            i2hT = pp.tile([128, 4 * H], BF16, tag="i2hT")
            h2hT = pp.tile([128, 4 * H], BF16, tag="h2hT")
            for wi, (src_d, dstT) in enumerate(((i2h_d, i2hT), (h2h_d, h2hT))):
                for r in range(4):
                    nat = wnat_pool.tile([128, 512], BF16, tag="wn512", name=f"wn{wi}_{r}")
                    nc.gpsimd.dma_start(nat[:], src_d[128 * r : 128 * (r + 1), :])
                    out_ap = dstT[:].rearrange("p (c f) -> p c f", c=4)[
                        :, :, 128 * r : 128 * (r + 1)
                    ]
                    nc.sync.dma_start(out_ap, nat[:], transpose=True)

            # w_ih^T main [128, 6144] (k<512), tail rows 0:97 (k=512:609); w_hh^T
            wihT_m = pp.tile([128, 4 * H3], BF16, tag="wihTm")
            wihT_t = pp.tile([128, 12 * 128], BF16, tag="wihTt")
            whhT = pp.tile([128, 4 * H3], BF16, tag="whhT")
            for r in range(12):
                nat = wnat_pool.tile([128, D + E], BF16, tag="wn640", name=f"wnih{r}")
                nc.gpsimd.dma_start(nat[:], wih_d[128 * r : 128 * (r + 1), :])
                out_ap = wihT_m[:].rearrange("p (c f) -> p c f", c=4)[
                    :, :, 128 * r : 128 * (r + 1)
                ]
                nc.sync.dma_start(out_ap, nat[:, 0:D], transpose=True)
                tp = ps.tile([128, 128], BF16, tag="tp")
                nc.tensor.transpose(tp[0:E, :], nat[:, D : D + E], eye_b[:, :])
                nc.vector.tensor_copy(wihT_t[0:E, 128 * r : 128 * (r + 1)], tp[0:E, :])
            for r in range(12):
                nat = wnat_pool.tile([128, 512], BF16, tag="wn512", name=f"wnhh{r}")
                nc.gpsimd.dma_start(nat[:], whh_d[128 * r : 128 * (r + 1), :])
                out_ap = whhT[:].rearrange("p (c f) -> p c f", c=4)[
                    :, :, 128 * r : 128 * (r + 1)
                ]
                nc.sync.dma_start(out_ap, nat[:], transpose=True)

            # prev^T [128, 128] bf16 (block kc at cols 32*kc); onehots^T rows 0:97
            prevT = pp.tile([128, 4 * BC], BF16, tag="prevT")
            for c in range(4):
                tp = ps.tile([128, 128], BF16, tag="tp")
                nc.tensor.transpose(
                    tp[0:128, 0:BC], prev_b[:, 128 * c : 128 * (c + 1)],
                    eye_b[0:BC, 0:BC],
                )
                nc.vector.tensor_copy(prevT[:, BC * c : BC * (c + 1)], tp[0:128, 0:BC])
            oneT = pp.tile([128, BC], BF16, tag="oneT")
            tp = ps.tile([128, 128], BF16, tag="tp")
            nc.tensor.transpose(tp[0:E, 0:BC], oneh_b[:, :], eye_b[0:BC, 0:BC])
            nc.vector.tensor_copy(oneT[0:E, :], tp[0:E, 0:BC])

            # ---------- prev_proj ([h, b] layout) with h2h_b bias ----------
            pproj = pp.tile([128, 4 * BC], F32, tag="pproj")
            for hc in range(4):
                acc = ps.tile([128, BC], F32, tag="pp")
                for kc in range(4):
                    nc.tensor.matmul(
                        acc[:, :],
                        h2hT[:, 512 * kc + 128 * hc : 512 * kc + 128 * (hc + 1)],
                        prevT[:, BC * kc : BC * (kc + 1)],
                        start=(kc == 0),
                        stop=(kc == 3),
                    )
                nc.scalar.activation(
                    pproj[:, BC * hc : BC * (hc + 1)],
                    acc[:, :],
                    AF.Identity,
                    bias=h2hb_pc[:, hc : hc + 1],
                )

            # ---------- main loop over b-pairs ----------
            exp_sb = pp.tile([BC, T], F32, tag="exp_sb")
            ctx_b = pp.tile([BC, D], BF16, tag="ctx_b")
            for j in range(NJ):
                # load 4 natural bands (rotating, cast f32->bf16 in DMA) and
                # xbar-transpose into the write-once bhT_j
                bhT_j = pp.tile([128, 4 * 512], BF16, tag=f"bht{j}", name=f"bht{j}")
                bands = []
                for u in range(4):
                    r = 4 * j + u
                    bhn = bhn_pool.tile([128, 512], BF16, tag=f"bhn{r}", name=f"bhn{r}")
                    nc.gpsimd.dma_start(bhn[:], bh_d[128 * r : 128 * (r + 1), :])
                    out_ap = bhT_j[:].rearrange("p (c f) -> p c f", c=4)[
                        :, :, 128 * u : 128 * (u + 1)
                    ]
                    nc.sync.dma_start(out_ap, bhn[:], transpose=True)
                    bands.append(bhn)

                # proj ([h, t]) + fused tanh(+prev_proj) + e = score . res
                e_ps = ps.tile([1, 512], F32, tag="e")
                for hc in range(4):
                    proj = ps.tile([128, 512], F32, tag="proj")
                    for kc in range(4):
                        nc.tensor.matmul(
                            proj[:, :],
                            i2hT[:, 512 * kc + 128 * hc : 512 * kc + 128 * (hc + 1)],
                            bhT_j[:, 512 * kc : 512 * (kc + 1)],
                            start=(kc == 0),
                            stop=(kc == 3),
                        )
                    res = res_pool.tile([128, 512], BF16, tag="res")
                    for half in range(2):
                        bl = 2 * j + half
                        nc.scalar.activation(
                            res[:, 256 * half : 256 * (half + 1)],
                            proj[:, 256 * half : 256 * (half + 1)],
                            AF.Tanh,
                            bias=pproj[:, BC * hc + bl : BC * hc + bl + 1],
                        )
                    nc.tensor.matmul(
                        e_ps[:, :],
                        scoreT[:, hc : hc + 1],
                        res[:, :],
                        start=(hc == 0),
                        stop=(hc == 3),
                        skip_group_check=True,
                    )

                # exp (no max subtraction; |e| small) -> [2, 256] rows
                erow = row_pool.tile([1, 512], F32, tag="erow")
                nc.scalar.activation(erow[:, :], e_ps[:, :], AF.Exp)
                for half in range(2):
                    nc.gpsimd.dma_start(
                        exp_sb[2 * j + half : 2 * j + half + 1, :],
                        erow[0:1, 256 * half : 256 * (half + 1)],
                    )
                e2 = soft_pool.tile([2, T], F32, tag="e2")
                for half in range(2):
                    nc.gpsimd.dma_start(
                        e2[half : half + 1, :],
                        erow[0:1, 256 * half : 256 * (half + 1)],
                    )

                # per-pair softmax scale (for context only; alpha output later)
                s2 = soft_pool.tile([2, 1], F32, tag="s2")
                nc.vector.reduce_sum(s2[:], e2[:, :], axis=AX.X)
                r2 = soft_pool.tile([2, 1], F32, tag="r2")
                nc.vector.reciprocal(r2[:], s2[:])
                a2 = soft_pool.tile([2, T], BF16, tag="a2")
                nc.vector.tensor_scalar_mul(a2[:], e2[:, :], r2[:])
                # alpha^T for this pair: [128, 4] (col 2*tc + half)
                aT = soft_pool.tile([128, 4], BF16, tag="aT")
                for tcb in range(2):
                    tp = ps.tile([128, 128], BF16, tag="tp")
                    nc.tensor.transpose(
                        tp[0:128, 0:2], a2[:, 128 * tcb : 128 * (tcb + 1)],
                        eye_b[0:2, 0:2],
                    )
                    nc.vector.tensor_copy(aT[:, 2 * tcb : 2 * (tcb + 1)], tp[0:128, 0:2])

                # context rows for both b of the pair
                for half in range(2):
                    b = 2 * j + half
                    cps = ps.tile([1, 512], F32, tag="e")
                    for tcb in range(2):
                        nc.tensor.matmul(
                            cps[:, :],
                            aT[:, 2 * tcb + half : 2 * tcb + half + 1],
                            bands[2 * half + tcb][:, :],
                            start=(tcb == 0),
                            stop=(tcb == 1),
                            skip_group_check=True,
                        )
                    crow = row_pool.tile([1, 512], BF16, tag="crow")
                    nc.scalar.copy(crow[:, :], cps[:, :])
                    nc.gpsimd.dma_start(ctx_b[b : b + 1, :], crow[:, :])

            # ---------- alpha output ----------
            sums = pp.tile([BC, 1], F32, tag="sums")
            nc.vector.reduce_sum(sums[:], exp_sb[:, :], axis=AX.X)
            recip = pp.tile([BC, 1], F32, tag="recip")
            nc.vector.reciprocal(recip[:], sums[:])
            alpha_f = pp.tile([BC, T], F32, tag="alpha_f")
            nc.vector.tensor_scalar_mul(alpha_f[:], exp_sb[:, :], recip[:])
            nc.gpsimd.dma_start(outa_d[:, :], alpha_f[:])

            # ---------- ctx^T [128, 128] (block kc at cols 32*kc) ----------
            ctxT = pp.tile([128, 4 * BC], BF16, tag="ctxT")
            for c in range(4):
                tp = ps.tile([128, 128], BF16, tag="tp")
                nc.tensor.transpose(
                    tp[0:128, 0:BC], ctx_b[:, 128 * c : 128 * (c + 1)],
                    eye_b[0:BC, 0:BC],
                )
                nc.vector.tensor_copy(ctxT[:, BC * c : BC * (c + 1)], tp[0:128, 0:BC])

            # ---------- GRU gates ([b, n] layout) ----------
            def gates_psum(w0, bias_rows, with_gi, with_gh):
                g = ps.tile([BC, 512], F32, tag="proj")
                for bi, brow in enumerate(bias_rows):
                    nc.tensor.matmul(
                        g[:, :], ones_b[:, :], brow, start=(bi == 0), stop=False,
                        skip_group_check=True,
                    )
                if with_gi:
                    for kc in range(4):
                        nc.tensor.matmul(
                            g[:, :],
                            ctxT[:, BC * kc : BC * (kc + 1)],
                            wihT_m[:, 1536 * kc + w0 : 1536 * kc + w0 + 512],
                            start=False, stop=False, skip_group_check=True,
                        )
                    nc.tensor.matmul(
                        g[:, :], oneT[0:E, :], wihT_t[0:E, w0 : w0 + 512],
                        start=False, stop=(not with_gh), skip_group_check=True,
                    )
                if with_gh:
                    for kc in range(4):
                        nc.tensor.matmul(
                            g[:, :],
                            prevT[:, BC * kc : BC * (kc + 1)],
                            whhT[:, 1536 * kc + w0 : 1536 * kc + w0 + 512],
                            start=False, stop=(kc == 3), skip_group_check=True,
                        )
                return g

            g_r = gates_psum(0, [bih_row[:, 0:512], bhh_row[:, 0:512]], True, True)
            r_sig = gate_pool.tile([BC, 512], F32, tag="g")
            nc.scalar.activation(r_sig[:], g_r[:, :], AF.Sigmoid)
            g_hn = gates_psum(1024, [bhh_row[:, 1024:1536]], False, True)
            rn = gate_pool.tile([BC, 512], F32, tag="g")
            nc.vector.tensor_mul(rn[:], r_sig[:], g_hn[:, :])
            g_in = gates_psum(1024, [bih_row[:, 1024:1536]], True, False)
            npre = gate_pool.tile([BC, 512], F32, tag="g")
            nc.vector.tensor_add(npre[:], g_in[:, :], rn[:])
            n_t = gate_pool.tile([BC, 512], F32, tag="g")
            nc.scalar.activation(n_t[:], npre[:], AF.Tanh)
            g_z = gates_psum(512, [bih_row[:, 512:1024], bhh_row[:, 512:1024]], True, True)
            z_sig = gate_pool.tile([BC, 512], F32, tag="g")
            nc.scalar.activation(z_sig[:], g_z[:, :], AF.Sigmoid)
            # cur = n + z*(prev - n)
            pmn = gate_pool.tile([BC, 512], F32, tag="g")
            nc.vector.tensor_sub(pmn[:], prev_f[:, :], n_t[:])
            zd = gate_pool.tile([BC, 512], F32, tag="g")
            nc.vector.tensor_mul(zd[:], z_sig[:], pmn[:])
            cur = gate_pool.tile([BC, 512], F32, tag="g")
            nc.vector.tensor_add(cur[:], n_t[:], zd[:])
            nc.gpsimd.dma_start(outh_d[:, :], cur[:])

    _hoist_excess_waits(nc)
    nc.finalize()
    return nc


# walrus wait-slot budgets per instruction type (empirical): the xbar
# transpose descriptor holds a single sync wait. Excess waits are hoisted
# onto a preceding same-engine Drain, which lowers to standalone wait
# commands and accepts many.
_WAIT_BUDGET = {
    "InstDmaTransposeAnt": 1,
    "InstDMACopy": 1,
    "InstTensorTensor": 1,
    "InstTensorScalarPtr": 1,
    "InstActivation": 1,
    "InstTensorCopy": 1,
    "InstTensorReduce": 1,
    "InstReciprocal": 1,
    "InstStreamTranspose": 1,
    "InstMemset": 1,
    "InstDrain": 1,
    "InstMatmult": 1,
    "InstLdweights": 1,
}
_DRAIN_WAITS = 1


def _hoist_excess_waits(nc):
    for fn in nc.m.functions:
        for blk in fn.blocks:
            new_insts = []
            for inst in blk.instructions:
                budget = _WAIT_BUDGET.get(type(inst).__name__)
                si = getattr(inst, "sync_info", None)
                if (
                    budget is not None
                    and si is not None
                    and si.on_wait
                    and len(si.on_wait) > budget
                ):
                    waits = list(si.on_wait)
                    keep = waits[:budget] if type(inst).__name__ != "InstDrain" else []
                    hoist = waits[budget:] if keep else waits
                    for i in range(0, len(hoist), _DRAIN_WAITS):
                        d = mybir.InstNoOp(
                            name=nc.get_next_instruction_name(),
                            ins=[],
                            outs=[],
                            text_hint="hoisted_wait",
                        )
                        d.engine = inst.engine
                        d.sync_info = mybir.SyncInfo(
                            on_wait=hoist[i : i + _DRAIN_WAITS], on_update=[]
                        )
                        nc.register_instruction(d)
                        new_insts.append(d)
                    inst.sync_info = mybir.SyncInfo(
                        on_wait=keep, on_update=list(si.on_update)
                    )
                new_insts.append(inst)
            try:
                blk.instructions[:] = new_insts
            except TypeError:
                blk.instructions = new_insts


_CACHE = {}


def _get_nc():
    if "nc" not in _CACHE:
        _CACHE["nc"] = build_nc()
    return _CACHE["nc"]


def make_in_maps(inputs):
    shared = {
        k: np.ascontiguousarray(np.asarray(inputs[k], dtype=np.float32))
        for k in (
            "i2h_w", "h2h_w", "h2h_b", "score_w",
            "rnn_w_ih", "rnn_b_ih", "rnn_w_hh", "rnn_b_hh",
        )
    }
    in_maps = []
    for c in range(NCORES):
        sl = slice(BC * c, BC * (c + 1))
        m = dict(shared)
        m["prev_hidden"] = np.ascontiguousarray(
            np.asarray(inputs["prev_hidden"], np.float32)[sl]
        )
        m["batch_H"] = np.ascontiguousarray(
            np.asarray(inputs["batch_H"], np.float32)[sl].reshape(NP, D)
        )
        m["char_onehots"] = np.ascontiguousarray(
            np.asarray(inputs["char_onehots"], np.float32)[sl]
        )
        in_maps.append(m)
    return in_maps


def kernel(**inputs):
    nc = _get_nc()
    res = run_bass_kernel_spmd(nc, make_in_maps(inputs), core_ids=list(range(NCORES)))
    results = res.results
    cur = np.concatenate([results[c]["out_hidden"] for c in range(NCORES)], axis=0)
    alpha = np.concatenate([results[c]["out_alpha"] for c in range(NCORES)], axis=0)
    return cur.astype(np.float32), alpha.reshape(B, 1, T).astype(np.float32)
